# revision 17
# baseline (speedup 1.0000x reference)
"""FP8Linear Trainium2 kernel.

Computes out = quant_e4m3(x) @ quant_e4m3(w).T in fp32, distributed over 8
NeuronCores as a 2x4 grid (x rows x w rows). Per core:

  x_shard [4096, 2048] f32, w_shard [2048, 2048] f32 -> out [4096, 2048] f32

Per-core pipeline:
  w: DMA load f32 -> PE transpose (fp32, identity matmul) -> PSUM ->
     ACT cast f32->fp8e4 into resident w.T tile [128, 16, 2048]
  x: DMA load f32 -> ACT cast f32->fp8 -> GpSimd copy fp8->bf16 ->
     DMA (ACT queue) to DRAM scratch -> DMA-transpose read [c-part, m] ->
     DVE cast bf16->fp8
  matmul: fp8 DoubleRow (pairs of 128-deep c-chunks), PSUM accumulate,
     N=512 tiles, ACT/DVE drain to SBUF, DMA out (ACT queue).

Queue discipline: input loads + transpose reads go on the SP (sync) HWDGE
FIFO and never wait on compute; compute-dependent stores (scratch, out) go
on the ACT HWDGE FIFO right after the op that produces their source.

TRN fp8e4 (max 240) matches OCP e4m3fn on [0, 240]; inputs are randn-scale
so the quantization grid is identical to the jax reference. Scales are
applied on the host (exact for any scale: round(x*s) then /(s_in*s_w)).
"""

import numpy as np
import ml_dtypes

# ---- problem constants (hardcoded per task contract) ----
A_DIM, B_DIM, C_DIM, OUT_DIM = 4, 2048, 2048, 8192
M_FULL = A_DIM * B_DIM  # 8192
GRID_M, GRID_O = 2, 4
N_CORES = GRID_M * GRID_O
M_CORE = M_FULL // GRID_M   # 4096
O_CORE = OUT_DIM // GRID_O  # 2048

P = 128


def build_nc(m_core=M_CORE, o_core=O_CORE, c_dim=C_DIM,
             m_chunk=512, n_tile=512, mm_psum_bufs=6):
    """Build the single-core Bass program (same program runs SPMD on 8 cores)."""
    import contextlib

    import concourse.bacc as bacc
    import concourse.mybir as mybir
    import concourse.tile as tile
    from concourse import masks

    f32 = mybir.dt.float32
    bf16 = mybir.dt.bfloat16
    fp8 = mybir.dt.float8e4
    Copy = mybir.ActivationFunctionType.Copy
    DR = mybir.MatmulPerfMode.DoubleRow

    S = c_dim // P              # c-chunks (16)
    assert S % 2 == 0 and S % 4 == 0
    SP = S // 2                 # DoubleRow pairs (8)
    MT = m_core // m_chunk      # x chunks (8)
    MW = m_chunk // P           # m windows per chunk (4)
    NT = o_core // n_tile       # o tiles (4)
    OC = o_core // P            # w o-chunks (16)
    MC = m_core // P            # x row-chunks (32)
    MCW = MW                    # row-chunks per x chunk

    nc = bacc.Bacc(None, target_bir_lowering=False, debug=False)
    x_in = nc.declare_dram_parameter("x_in", [m_core, c_dim], f32, isOutput=False)
    w_in = nc.declare_dram_parameter("w_in", [o_core, c_dim], f32, isOutput=False)
    out = nc.declare_dram_parameter("out", [m_core, o_core], f32, isOutput=True)

    with tile.TileContext(nc) as tc:
        with contextlib.ExitStack() as ctx:
            dram = ctx.enter_context(tc.tile_pool(name="dram", bufs=1, space="DRAM"))
            const = ctx.enter_context(tc.tile_pool(name="const", bufs=1))
            wstg = ctx.enter_context(tc.tile_pool(name="wstg", bufs=3))
            wres = ctx.enter_context(tc.tile_pool(name="wres", bufs=1))
            wtp = ctx.enter_context(tc.tile_pool(name="wtp", bufs=2, space="PSUM"))
            xstg = ctx.enter_context(tc.tile_pool(name="xstg", bufs=4))
            x8p = ctx.enter_context(tc.tile_pool(name="x8p", bufs=4))
            xtb = ctx.enter_context(tc.tile_pool(name="xtb", bufs=2))
            xtf = ctx.enter_context(tc.tile_pool(name="xtf", bufs=3))
            mmp = ctx.enter_context(
                tc.tile_pool(name="mmp", bufs=mm_psum_bufs, space="PSUM"))
            osb = ctx.enter_context(tc.tile_pool(name="osb", bufs=2))

            xbf = dram.tile([m_core, c_dim], bf16)  # e4m3-grid scratch

            identity = const.tile([P, P], bf16)
            masks.make_identity(nc, identity[:])

            # resident w.T as fp8: [c-part, c-chunk, o]
            WT = wres.tile([P, S, o_core], fp8)

            def x_chain(mc):
                """x rows [mc*128, +128): load, quantize, stage to scratch.
                The fp8->bf16 upcast happens inside the SWDGE DMA."""
                x_stage = xstg.tile([P, c_dim], f32, tag="x_stage", name="x_stage")
                nc.gpsimd.dma_start(out=x_stage[:], in_=x_in[mc * P:(mc + 1) * P, :])
                x8 = x8p.tile([P, c_dim], fp8, tag="x8", name="x8")
                nc.scalar.activation(x8[:], x_stage[:], Copy)
                nc.gpsimd.dma_start(out=xbf[mc * P:(mc + 1) * P, :], in_=x8[:])

            def w_chunk(oc):
                """w rows [oc*128, +128): load, quantize to fp8, upcast bf16,
                PE-transpose, land in resident WT (DVE drains the psum)."""
                w_stage = wstg.tile([P, c_dim], f32, tag="w_stage", name="w_stage")
                nc.sync.dma_start(out=w_stage[:], in_=w_in[oc * P:(oc + 1) * P, :])
                w8 = wstg.tile([P, c_dim], fp8, tag="w8", name="w8")
                nc.scalar.activation(w8[:], w_stage[:], Copy)
                wb = wstg.tile([P, c_dim], bf16, tag="wb", name="wb")
                nc.vector.tensor_copy(out=wb[:], in_=w8[:])
                for g in range(S // 4):
                    pst = wtp.tile([P, 4, P], bf16, tag="wt_psum", name="wt_psum")
                    for j in range(4):
                        s = 4 * g + j
                        nc.tensor.transpose(
                            pst[:, j, :], wb[:, s * P:(s + 1) * P], identity[:])
                    nc.vector.tensor_copy(
                        out=WT[:, 4 * g:4 * g + 4, oc * P:(oc + 1) * P], in_=pst[:])

            def x_transpose(mt):
                m0 = mt * m_chunk
                XTb = xtb.tile([P, S, m_chunk], bf16, tag="XTb", name="XTb")
                for s in range(S):
                    nc.sync.dma_start_transpose(
                        out=XTb[:, s, :],
                        in_=xbf[m0:m0 + m_chunk, s * P:(s + 1) * P])
                XT = xtf.tile([P, S, m_chunk], fp8, tag="XT", name="XT")
                nc.vector.tensor_copy(out=XT[:], in_=XTb[:])
                return XT

            def x_matmul(mt, XT):
                m0 = mt * m_chunk
                for mw in range(MW):
                    ps_tiles = [
                        mmp.tile([P, n_tile], f32, tag="mm_psum", name="mm_psum")
                        for _ in range(NT)]
                    for sp in range(SP):
                        lhsT = XT[:, 2 * sp:2 * sp + 2, mw * P:(mw + 1) * P]
                        for nt in range(NT):
                            nc.tensor.matmul(
                                ps_tiles[nt][:],
                                lhsT,
                                WT[:, 2 * sp:2 * sp + 2,
                                   nt * n_tile:(nt + 1) * n_tile],
                                start=(sp == 0), stop=(sp == SP - 1),
                                perf_mode=DR)
                    ot = osb.tile([P, o_core], f32, tag="ot", name="ot")
                    for nt in range(NT):
                        dst = ot[:, nt * n_tile:(nt + 1) * n_tile]
                        nc.vector.tensor_copy(out=dst, in_=ps_tiles[nt][:])
                    nc.scalar.dma_start(
                        out=out[m0 + mw * P:m0 + (mw + 1) * P, :], in_=ot[:])

            # ---- wave 0: x chunk-0 loads first, then w prep with x chunk-1
            # interleaved; transpose chunk 0 mid-w, chunk 1 after ----
            ahead = 2  # load-chain prefetch distance (chunks)
            for mc in range(min(MCW, MC)):
                x_chain(mc)
            xts = {}
            for oc in range(OC):
                if oc < min(MCW, MC - MCW):
                    x_chain(MCW + oc)
                w_chunk(oc)
                if oc == OC // 2 - 1:
                    xts[0] = x_transpose(0)
            if MT > 1:
                xts[1] = x_transpose(1)

            # ---- matmul waves: transpose chunk mt+2, load chunk mt+ahead,
            # compute chunk mt ----
            for mt in range(MT):
                for mc in range(MCW * (mt + ahead), min(MCW * (mt + ahead + 1), MC)):
                    x_chain(mc)
                if mt + 2 < MT:
                    xts[mt + 2] = x_transpose(mt + 2)
                x_matmul(mt, xts.pop(mt))

    nc.finalize()
    return nc


_NC = None


def _get_nc():
    global _NC
    if _NC is None:
        _NC = build_nc()
    return _NC


def kernel(input, weight, input_scale_e4m3=None, weight_scale_e4m3=None,
           **_unused):
    from concourse.bass_utils import run_bass_kernel_spmd

    x = np.asarray(input, dtype=np.float32).reshape(M_FULL, C_DIM)
    w = np.asarray(weight, dtype=np.float32)
    s_in = float(np.asarray(input_scale_e4m3)) if input_scale_e4m3 is not None else 1.0
    s_w = float(np.asarray(weight_scale_e4m3)) if weight_scale_e4m3 is not None else 1.0

    # reference semantics: round(x*s)/s etc.; fold scales on host (exact)
    if s_in != 1.0:
        x = x * s_in
    if s_w != 1.0:
        w = w * s_w

    nc = _get_nc()
    in_maps = []
    for mi in range(GRID_M):
        for oj in range(GRID_O):
            in_maps.append({
                "x_in": x[mi * M_CORE:(mi + 1) * M_CORE],
                "w_in": w[oj * O_CORE:(oj + 1) * O_CORE],
            })
    res = run_bass_kernel_spmd(nc, in_maps, core_ids=list(range(N_CORES)))

    out = np.empty((M_FULL, OUT_DIM), np.float32)
    for k, r in enumerate(res.results):
        mi, oj = divmod(k, GRID_O)
        out[mi * M_CORE:(mi + 1) * M_CORE, oj * O_CORE:(oj + 1) * O_CORE] = r["out"]

    inv = 1.0 / (s_in * s_w)
    if inv != 1.0:
        out = out * inv
    return out.reshape(A_DIM, B_DIM, OUT_DIM)
